# revision 26
# baseline (speedup 1.0000x reference)
"""Causal self-attention block (B=4, T=2048, C=2048, H=16, D=128) on 8 trn2 cores.

Sharding: tensor-parallel over head groups (2 groups of 8 heads) x
data-parallel over batch (4). Core (g, b) computes, for batch b and heads
[8g, 8g+8): qkv projection, causal attention, and the partial output
projection contribution attn_out[:, heads_g] @ Wproj[rows_g]. The host sums
the two partial yT per batch, adds bproj, and transposes back.

v3 (active, build_nc_v3): phase 2 runs c-outer/h-inner with phase 3
interleaved two output tiles per head-slot, so the PE never drains at a
phase boundary. V is produced in NATURAL [t,d] layout directly in phase 1
(128-wide matmuls at equal PE cost; v-bias folded into bproj on the host
since softmax rows sum to 1), which removes all per-slot PE transposes.
Softmax colsums are per-block ones-matmuls on the PE (no DVE chains);
reciprocal broadcast via Pool partition_broadcast (no DRAM round-trip;
source slice must sit at partition base 0). Off-diagonal S blocks are
computed in [128,2,512] psum pairs and exp'd with ONE Act instruction per
pair — Act per-instruction overhead, not exp payload, was the phase-2
binder. qkvT DRAM tiles are double-buffered by rep parity and v-tiles are
emitted last in phase 1, so rep r+1's q/k projection overlaps rep r's
attention (~-138us/rep in the slope bench). PSUM start=True resets the
whole bank: never interleave accumulation groups in one bank.

v2 (kept as _BUILD_V2): h-outer phases, DVE colsum chains, DRAM-round-trip
reciprocal broadcast, PE transposes of V^T per chunk.
"""

import sys

sys.path.insert(0, "/opt/trn_rl_repo")

import numpy as np

import concourse.bass as bass
import concourse.mybir as mybir
import concourse.tile as tile
from concourse import bacc
from concourse.bass_utils import run_bass_kernel_spmd
from concourse.masks import make_identity

F32 = mybir.dt.float32
F32R = mybir.dt.float32r
BF16 = mybir.dt.bfloat16
AF = mybir.ActivationFunctionType
MUL = mybir.AluOpType.mult

B, T, C = 4, 2048, 2048
H, D = 16, 128
G = 2  # head-group shards
HPC = H // G  # heads per core = 8
CT = C // 128  # contraction chunks = 16
NT = T // 512  # t chunks of 512 = 4
NJ = 3 * HPC  # qkv col tiles per core = 24
SCALE = 1.0 / float(np.sqrt(D))
# off-diagonal blocks per chunk handed to the Pool accumulation chain
POOL_BLOCKS = {0: 0, 1: 2, 2: 4, 3: 6}
SUM_CHAINS = "dve"  # 'dve_pool' | 'dve' (no Pool adds) | 'pe' (baseline)
BCAST = "dram"  # 'pool' (partition_broadcast op) | 'dram' (DMA round-trip)
QKV_SBUF = False  # keep qkvT in SBUF (no DRAM round-trip between phases 1/2)


def build_nc(phases=(1, 2, 3), reps=1):
    nc = bacc.Bacc("TRN2", target_bir_lowering=False)
    xT = nc.dram_tensor("xT", [128, CT, T], BF16, kind="ExternalInput")
    wqkv = nc.dram_tensor("wqkv", [128, NJ, CT, 128], BF16, kind="ExternalInput")
    wproj = nc.dram_tensor("wproj", [128, CT, HPC, 128], BF16, kind="ExternalInput")
    bqkv = nc.dram_tensor("bqkv", [128, NJ], F32, kind="ExternalInput")
    tri_in = nc.dram_tensor("tri", [128, 128], BF16, kind="ExternalInput")
    yT = nc.dram_tensor("yT", [C, T], F32, kind="ExternalOutput")
    yT_r = yT.rearrange("(i p) t -> p i t", p=128)

    with tile.TileContext(nc) as tc:
        with (
            tc.tile_pool(name="const", bufs=1) as cst,
            tc.tile_pool(name="dram", bufs=1, space="DRAM") as dram,
        ):
            # allocate constants up front; their loads are emitted after the
            # warm-start DMAs so the first matmul's deps go first in the queue
            tri_sb = cst.tile([128, 128], BF16)
            bias_sb = cst.tile([128, NJ], F32)
            ident = cst.tile([128, 128], BF16)
            ones_f = cst.tile([128, 1], F32)
            ones = cst.tile([128, 1], F32R)
            ones_bf = cst.tile([128, 1], BF16)

            def load_consts():
                nc.sync.dma_start(tri_sb, tri_in.ap())
                nc.sync.dma_start(bias_sb, bqkv.ap())
                make_identity(nc, ident)
                nc.vector.memset(ones_f, 1.0)
                nc.vector.tensor_copy(ones, ones_f)
                nc.vector.tensor_copy(ones_bf, ones_f)

            if not QKV_SBUF:
                qkvT = [
                    dram.tile([128, T], BF16, name=f"qkvT{j}", tag=f"qkvT{j}")
                    for j in range(NJ)
                ]

            if 1 not in phases:
                load_consts()

            from contextlib import ExitStack

            for _rep in range(reps):
              with ExitStack() as rep_ctx:
                  if QKV_SBUF:
                    qkvp = rep_ctx.enter_context(
                        tc.tile_pool(name=f"qkv_{_rep}", bufs=1)
                      )
                    qkvT = [
                        qkvp.tile([128, T], BF16, name=f"qkvsb{j}_{_rep}")
                          for j in range(NJ)
                      ]
                  # ---------------- phase 1: qkvT[col, t] = W^T x^T (+bias) -------
                  if 1 in phases:
                   with (
                      tc.tile_pool(name=f"p1x_{_rep}", bufs=1) as p1x,
                      tc.tile_pool(name=f"p1w_{_rep}", bufs=3) as p1w,
                      tc.tile_pool(name=f"p1s_{_rep}", bufs=4) as p1s,
                      tc.tile_pool(name=f"ps1_{_rep}", bufs=8, space="PSUM") as ps1,
                  ):
                      # interleave q/k/v col-tiles so head h's three tensors are all
                      # ready after 3*(h+1) of the 24 tiles
                      j_order = [base + h for h in range(HPC) for base in (0, HPC, 2 * HPC)]
                      WARM = 2  # first j's run chunk-outer to overlap the xs load
                      warm_w = {}
                      for wj in j_order[:WARM]:
                          w_sb = p1w.tile([128, CT, 128], BF16, tag="w")
                          nc.sync.dma_start(w_sb, wqkv[:, wj])
                          warm_w[wj] = w_sb
                      xs = p1x.tile([128, CT, T], BF16)
                      for cc in range(CT):
                          nc.sync.dma_start(xs[:, cc, :], xT[:, cc, :])
                      if _rep == 0:
                          load_consts()
                      # warm-up: 8 psum groups accumulate chunk-by-chunk as the xs
                      # chunks arrive, so PE works during the x load
                      warm_ps = {
                          (wj, c): ps1.tile(
                              [128, 512], F32, tag="ps", name=f"warm_ps_{wj}_{c}"
                          )
                          for wj in j_order[:WARM]
                          for c in range(NT)
                      }
                      for cc in range(CT):
                          for wj in j_order[:WARM]:
                              for c in range(NT):
                                  nc.tensor.matmul(
                                      warm_ps[(wj, c)],
                                      lhsT=warm_w[wj][:, cc, :],
                                      rhs=xs[:, cc, 512 * c : 512 * (c + 1)],
                                      start=(cc == 0),
                                      stop=(cc == CT - 1),
                                  )
                      for wj in j_order[:WARM]:
                          for c in range(NT):
                              if QKV_SBUF:
                                  nc.vector.tensor_scalar_add(
                                      qkvT[wj][:, 512 * c : 512 * (c + 1)],
                                      warm_ps[(wj, c)],
                                      bias_sb[:, wj : wj + 1],
                                  )
                              else:
                                  st = p1s.tile([128, 512], BF16, tag="st")
                                  nc.vector.tensor_scalar_add(
                                      st, warm_ps[(wj, c)], bias_sb[:, wj : wj + 1]
                                  )
                                  nc.sync.dma_start(
                                      qkvT[wj][:, 512 * c : 512 * (c + 1)], st
                                  )
                      for j in j_order[WARM:]:
                          w_sb = p1w.tile([128, CT, 128], BF16, tag="w")
                          nc.sync.dma_start(w_sb, wqkv[:, j])
                          for c in range(NT):
                              ps = ps1.tile([128, 512], F32, tag="ps")
                              for cc in range(CT):
                                  nc.tensor.matmul(
                                      ps,
                                      lhsT=w_sb[:, cc, :],
                                      rhs=xs[:, cc, 512 * c : 512 * (c + 1)],
                                      start=(cc == 0),
                                      stop=(cc == CT - 1),
                                  )
                              if QKV_SBUF:
                                  nc.vector.tensor_scalar_add(
                                      qkvT[j][:, 512 * c : 512 * (c + 1)],
                                      ps,
                                      bias_sb[:, j : j + 1],
                                  )
                              else:
                                  st = p1s.tile([128, 512], BF16, tag="st")
                                  nc.vector.tensor_scalar_add(
                                      st, ps, bias_sb[:, j : j + 1]
                                  )
                                  nc.sync.dma_start(
                                      qkvT[j][:, 512 * c : 512 * (c + 1)], st
                                  )

                  # phase 0: memset ot_tiles (phase-3 isolation benchmark mode)
                  if 0 in phases:
                   with tc.tile_pool(name=f"otp_{_rep}", bufs=1) as otp:
                    ot_tiles = {}
                    for h in range(HPC):
                        for c in range(NT):
                            ot = otp.tile(
                                [128, 512], BF16, name=f"ot_{h}_{c}", tag=f"ot_{h}_{c}"
                            )
                            nc.vector.memset(ot, 0.01)
                            ot_tiles[(h, c)] = ot
                    if 3 in phases:
                     with (
                        tc.tile_pool(name=f"p3w_{_rep}", bufs=3) as p3w,
                        tc.tile_pool(name=f"p3y_{_rep}", bufs=4) as p3y,
                        tc.tile_pool(name=f"ps3_{_rep}", bufs=4, space="PSUM") as ps3,
                     ):
                      for i in range(CT):
                          wp = p3w.tile([128, HPC, 128], BF16, tag="wp")
                          nc.sync.dma_start(wp, wproj[:, i])
                          for c in range(NT):
                              ps_y = ps3.tile([128, 512], F32, tag="y")
                              for hh in range(HPC):
                                  nc.tensor.matmul(
                                      ps_y,
                                      lhsT=wp[:, hh, :],
                                      rhs=ot_tiles[(hh, c)][:],
                                      start=(hh == 0),
                                      stop=(hh == HPC - 1),
                                  )
                              ys = p3y.tile([128, 512], F32, tag="ys")
                              nc.scalar.copy(ys, ps_y)
                              nc.sync.dma_start(yT_r[:, i, 512 * c : 512 * (c + 1)], ys)

                  # ---- phases 2+3 share a persistent SBUF pool holding the
                  # attention outputs (no DRAM round-trip, no phase-3 reload) ----
                  if 2 in phases:
                   with tc.tile_pool(name=f"otp_{_rep}", bufs=1) as otp:
                    ot_tiles = {}
                    with (
                      tc.tile_pool(name=f"p2qk_{_rep}", bufs=2) as p2qk,
                      tc.tile_pool(name=f"p2v_{_rep}", bufs=2) as p2v,
                      tc.tile_pool(name=f"p2p_{_rep}", bufs=3) as p2p,
                      tc.tile_pool(name=f"p2sc_{_rep}", bufs=4) as p2sc,
                      tc.tile_pool(name=f"ps2s_{_rep}", bufs=3, space="PSUM") as ps2s,
                      tc.tile_pool(name=f"ps2t_{_rep}", bufs=2, space="PSUM") as ps2t,
                      tc.tile_pool(name=f"ps2m_{_rep}", bufs=m_bufs, space="PSUM") as ps2m,
                      tc.tile_pool(name=f"ps2o_{_rep}", bufs=2, space="PSUM") as ps2o,
                      tc.tile_pool(name=f"dram_rb_{_rep}", bufs=4, space="DRAM") as dram_rb,
                    ):
                      for h in range(HPC):
                          if QKV_SBUF:
                              q_sb = qkvT[h]
                              k_sb = qkvT[HPC + h]
                              vt_sb = qkvT[2 * HPC + h]
                          else:
                              q_sb = p2qk.tile([128, T], BF16, tag="q")
                              nc.sync.dma_start(q_sb, qkvT[h][:])
                              k_sb = p2qk.tile([128, T], BF16, tag="k")
                              nc.sync.dma_start(k_sb, qkvT[HPC + h][:])
                              vt_sb = p2qk.tile([128, T], BF16, tag="vt")
                              nc.sync.dma_start(vt_sb, qkvT[2 * HPC + h][:])

                          # V natural layout via PE transposes of V^T blocks,
                          # spread per chunk (chunk c's AV needs blocks <= 4c+3)
                          v_sb = p2v.tile([128, T // 128, 128], BF16, tag="v")

                          tri = tri_sb
                          for c in range(NT):
                              for jb in range(4 * c, 4 * c + 4):
                                  ps_v = ps2t.tile([128, 128], BF16, tag="pst")
                                  nc.tensor.transpose(
                                      ps_v, vt_sb[:, 128 * jb : 128 * (jb + 1)], ident
                                  )
                                  nc.vector.tensor_copy(v_sb[:, jb, :], ps_v)

                              nblk = 4 * c + 4
                              pta = p2p.tile(
                                  [128, 8, 512], BF16, tag="pT", name=f"pta_{h}_{c}"
                              )
                              ptb = (
                                  p2p.tile(
                                      [128, 8, 512], BF16, tag="pT", name=f"ptb_{h}_{c}"
                                  )
                                  if nblk > 8
                                  else None
                              )

                              def pT(j):
                                  return (pta if j < 8 else ptb)[:, j % 8, :]

                              for j in range(nblk):
                                  v = j - 4 * c  # >= 0 on diagonal-group blocks
                                  off = 128 * v if v > 0 else 0
                                  ps_s = ps2s.tile([128, 512], F32, tag="s")
                                  nc.tensor.matmul(
                                      ps_s[:, off:512],
                                      lhsT=k_sb[:, 128 * j : 128 * (j + 1)],
                                      rhs=q_sb[:, 512 * c + off : 512 * (c + 1)],
                                      start=True,
                                      stop=True,
                                  )
                                  # exp only over the causally-reachable columns;
                                  # columns < off are never read downstream.
                                  nc.scalar.activation(
                                      pT(j)[:, off:512], ps_s[:, off:512], AF.Exp,
                                      scale=SCALE,
                                  )
                                  if v >= 0:
                                      nc.vector.tensor_mul(
                                          pT(j)[:, off : off + 128],
                                          pT(j)[:, off : off + 128],
                                          tri,
                                      )

                              # ---- softmax denominators ----
                              accA = accB = None
                              if SUM_CHAINS != "pe":
                                  # accumulation chains (DVE + optionally Pool),
                                  # finished by matmuls into one [1,512] group
                                  npool = POOL_BLOCKS[c] if SUM_CHAINS == "dve_pool" else 0
                                  b_list = list(range(npool))  # off-diag, full
                                  a_off = list(range(npool, 4 * c))  # off-diag rest
                                  accA = p2sc.tile(
                                      [128, 512], F32R, tag="acc", name=f"accA_{h}_{c}"
                                  )
                                  if a_off:
                                      # init with two full-width blocks
                                      nc.vector.tensor_add(accA, pT(4 * c), pT(a_off[0]))
                                      for j in a_off[1:]:
                                          nc.vector.tensor_add(accA, accA, pT(j))
                                  else:
                                      nc.vector.tensor_copy(accA, pT(4 * c))
                                  for v in range(1, 4):
                                      off = 128 * v
                                      nc.vector.tensor_add(
                                          accA[:, off:512],
                                          accA[:, off:512],
                                          pT(4 * c + v)[:, off:512],
                                      )
                                  if npool:
                                      accB = p2sc.tile(
                                          [128, 512], F32R, tag="acc",
                                          name=f"accB_{h}_{c}",
                                      )
                                      nc.gpsimd.tensor_add(
                                          accB, pT(b_list[0]), pT(b_list[1])
                                      )
                                      for j in b_list[2:]:
                                          nc.gpsimd.tensor_add(accB, accB, pT(j))

                              # ---- out^T = V P^T (before the ones-matmuls so
                              # the PE never waits on the DVE/Pool chains) ----
                              ps_o = ps2o.tile([128, 512], F32, tag="o")
                              for j in range(nblk):
                                  v = j - 4 * c
                                  off = 128 * v if v > 0 else 0
                                  nc.tensor.matmul(
                                      ps_o[:, off:512],
                                      lhsT=v_sb[:, j, :],
                                      rhs=pT(j)[:, off:512],
                                      start=(j == 0),
                                      stop=(j == nblk - 1),
                                  )

                              ps_sum = ps2m.tile([1, 512], F32, tag="sum")
                              if SUM_CHAINS == "pe":
                                  for j in range(nblk):
                                      v = j - 4 * c
                                      off = 128 * v if v > 0 else 0
                                      nc.tensor.matmul(
                                          ps_sum[:, off:512],
                                          lhsT=ones_bf,
                                          rhs=pT(j)[:, off:512],
                                          start=(j == 0),
                                          stop=(j == nblk - 1),
                                      )
                              else:
                                  nc.tensor.matmul(
                                      ps_sum, lhsT=ones, rhs=accA,
                                      start=True, stop=(accB is None),
                                  )
                                  if accB is not None:
                                      nc.tensor.matmul(
                                          ps_sum, lhsT=ones, rhs=accB,
                                          start=False, stop=True,
                                      )
                              rs = p2sc.tile([1, 512], F32, tag="rs")
                              nc.vector.reciprocal(rs, ps_sum[0:1, :])
                              rb = p2sc.tile([128, 512], F32, tag="rb")
                              if BCAST == "pool":
                                  nc.gpsimd.partition_broadcast(rb, rs)
                              else:
                                  rbx = dram_rb.tile([1, 512], F32, tag="rbx")
                                  nc.sync.dma_start(rbx, rs)
                                  nc.gpsimd.dma_start(
                                      rb, rbx[0].partition_broadcast(128)
                                  )

                              ot = otp.tile(
                                  [128, 512], BF16, name=f"ot_{h}_{c}", tag=f"ot_{h}_{c}"
                              )
                              nc.vector.tensor_tensor(ot, ps_o, rb, MUL)
                              ot_tiles[(h, c)] = ot

                    # -------- phase 3: yT = Wproj_g^T attn_outT (from SBUF) -------
                    if 3 in phases:
                     with (
                        tc.tile_pool(name=f"p3w_{_rep}", bufs=3) as p3w,
                        tc.tile_pool(name=f"p3y_{_rep}", bufs=4) as p3y,
                        tc.tile_pool(name=f"ps3_{_rep}", bufs=4, space="PSUM") as ps3,
                     ):
                      for i in range(CT):
                          wp = p3w.tile([128, HPC, 128], BF16, tag="wp")
                          nc.sync.dma_start(wp, wproj[:, i])
                          for c in range(NT):
                              ps_y = ps3.tile([128, 512], F32, tag="y")
                              for hh in range(HPC):
                                  nc.tensor.matmul(
                                      ps_y,
                                      lhsT=wp[:, hh, :],
                                      rhs=ot_tiles[(hh, c)][:],
                                      start=(hh == 0),
                                      stop=(hh == HPC - 1),
                                  )
                              ys = p3y.tile([128, 512], F32, tag="ys")
                              nc.scalar.copy(ys, ps_y)
                              nc.sync.dma_start(yT_r[:, i, 512 * c : 512 * (c + 1)], ys)

    nc.compile()
    return nc


def build_nc_v3(phases=(1, 2, 3), reps=1, bcast="dram", pp_bufs=2, ld_bufs=3, diag_pair=False, probe=None, o_bufs=2, m_bufs=1, p3_bufs=1, ot_lag=1):
    """c-outer/h-inner phases 2+3, interleaved per chunk.

    vs v2: softmax colsums via per-block ones-matmuls on the PE (no DVE
    accumulation chains), reciprocal broadcast on Pool (no DRAM round-trip),
    Act engine runs ONLY Exp (no activation-table swaps), phase 3 runs per
    chunk right after the 8 heads' ot tiles for that chunk are ready (PE
    keeps streaming instead of draining at the phase boundary), phase-3
    psum->sbuf copies moved to DVE.
    """
    nc = bacc.Bacc("TRN2", target_bir_lowering=False)
    xT = nc.dram_tensor("xT", [128, CT, T], BF16, kind="ExternalInput")
    wqkv = nc.dram_tensor("wqkv", [128, NJ, CT, 128], BF16, kind="ExternalInput")
    wproj = nc.dram_tensor("wproj", [128, CT, HPC, 128], BF16, kind="ExternalInput")
    bqkv = nc.dram_tensor("bqkv", [128, NJ], F32, kind="ExternalInput")
    tri_in = nc.dram_tensor("tri", [128, 128], BF16, kind="ExternalInput")
    yT = nc.dram_tensor("yT", [C, T], F32, kind="ExternalOutput")
    yT_r = yT.rearrange("(i p) t -> p i t", p=128)

    with tile.TileContext(nc) as tc:
        with (
            tc.tile_pool(name="const", bufs=1) as cst,
            tc.tile_pool(name="dram", bufs=1, space="DRAM") as dram,
        ):
            tri_sb = cst.tile([128, 128], BF16)
            bias_sb = cst.tile([128, NJ], F32)
            ident = cst.tile([128, 128], BF16)
            ones_f = cst.tile([128, 1], F32)
            ones_bf = cst.tile([128, 1], BF16)
            ones_row = cst.tile([1, 128], BF16)

            def load_consts():
                nc.vector.memset(ones_row, 1.0)
                nc.sync.dma_start(tri_sb, tri_in.ap())
                nc.sync.dma_start(bias_sb, bqkv.ap())
                make_identity(nc, ident)
                nc.vector.memset(ones_f, 1.0)
                nc.vector.tensor_copy(ones_bf, ones_f)

            qkvT_db = [
                [
                    dram.tile([128, T], BF16, name=f"qkvT{j}_{p}", tag=f"qkvT{j}_{p}")
                    for j in range(NJ)
                ]
                for p in range(2)
            ]

            if 1 not in phases:
                load_consts()

            for _rep in range(reps):
                qkvT = qkvT_db[_rep % 2]
                # persistent across phases: K residents, V natural, Wproj
                kvp_ctx = tc.tile_pool(name=f"kv_{_rep}", bufs=1)
                kvp = kvp_ctx.__enter__()
                k_res = {
                    h: kvp.tile([128, T], BF16, name=f"kres_{h}_{_rep}")
                    for h in range(HPC)
                }
                v_sb = {
                    h: kvp.tile([128, T], BF16, name=f"vsb_{h}_{_rep}")
                    for h in range(HPC)
                }
                wp_all = kvp.tile([128, CT, HPC, 128], BF16, name=f"wpall_{_rep}")

                # ---------------- phase 1: qkvT[col, t] = W^T x^T (+bias) -----
                if 1 in phases:
                    with (
                        tc.tile_pool(name=f"p1x_{_rep}", bufs=1) as p1x,
                        tc.tile_pool(name=f"p1w_{_rep}", bufs=3) as p1w,
                        tc.tile_pool(name=f"p1s_{_rep}", bufs=4) as p1s,
                        tc.tile_pool(name=f"ps1_{_rep}", bufs=8, space="PSUM") as ps1,
                    ):
                        j_order = [
                            base + h for h in range(HPC) for base in (0, HPC)
                        ] + [2 * HPC + h for h in range(HPC)]
                        WARM = 2
                        warm_w = {}
                        for wj in j_order[:WARM]:
                            w_sb = p1w.tile([128, CT, 128], BF16, tag="w")
                            nc.sync.dma_start(w_sb, wqkv[:, wj])
                            warm_w[wj] = w_sb
                        xs = p1x.tile([128, CT, T], BF16)
                        for cc in range(CT):
                            nc.sync.dma_start(xs[:, cc, :], xT[:, cc, :])
                        if _rep == 0:
                            load_consts()
                        warm_ps = {
                            (wj, c): ps1.tile(
                                [128, 512], F32, tag="ps", name=f"warm_ps_{wj}_{c}"
                            )
                            for wj in j_order[:WARM]
                            for c in range(NT)
                        }
                        for cc in range(CT):
                            for wj in j_order[:WARM]:
                                for c in range(NT):
                                    nc.tensor.matmul(
                                        warm_ps[(wj, c)],
                                        lhsT=warm_w[wj][:, cc, :],
                                        rhs=xs[:, cc, 512 * c : 512 * (c + 1)],
                                        start=(cc == 0),
                                        stop=(cc == CT - 1),
                                    )
                        for wj in j_order[:WARM]:
                            for c in range(NT):
                                st = p1s.tile([128, 512], BF16, tag="st")
                                nc.vector.tensor_scalar_add(
                                    st, warm_ps[(wj, c)], bias_sb[:, wj : wj + 1]
                                )
                                nc.sync.dma_start(
                                    qkvT[wj][:, 512 * c : 512 * (c + 1)], st
                                )
                        for j in j_order[WARM:]:
                            w_sb = p1w.tile([128, CT, 128], BF16, tag="w")
                            nc.sync.dma_start(w_sb, wqkv[:, j])
                            if j >= 2 * HPC:
                                # V in natural layout [t, d], written straight
                                # into the persistent v_sb (bias folded into
                                # bproj on the host; softmax rows sum to 1)
                                vh = j - 2 * HPC
                                for tb in range(CT):
                                    ps = ps1.tile([128, 512], F32, tag="ps")
                                    for cc in range(CT):
                                        nc.tensor.matmul(
                                            ps[:, 0:128],
                                            lhsT=xs[:, cc, 128 * tb : 128 * (tb + 1)],
                                            rhs=w_sb[:, cc, :],
                                            start=(cc == 0),
                                            stop=(cc == CT - 1),
                                        )
                                    nc.vector.tensor_copy(
                                        v_sb[vh][:, 128 * tb : 128 * (tb + 1)],
                                        ps[:, 0:128],
                                    )
                                continue
                            for c in range(NT):
                                ps = ps1.tile([128, 512], F32, tag="ps")
                                for cc in range(CT):
                                    nc.tensor.matmul(
                                        ps,
                                        lhsT=w_sb[:, cc, :],
                                        rhs=xs[:, cc, 512 * c : 512 * (c + 1)],
                                        start=(cc == 0),
                                        stop=(cc == CT - 1),
                                    )
                                st = p1s.tile([128, 512], BF16, tag="st")
                                nc.vector.tensor_scalar_add(
                                    st, ps, bias_sb[:, j : j + 1]
                                )
                                nc.sync.dma_start(
                                    qkvT[j][:, 512 * c : 512 * (c + 1)], st
                                )

                # ------------- phases 2+3: c-outer, phase 3 per chunk --------
                if 2 in phases:
                    with (
                        tc.tile_pool(name=f"ld_{_rep}", bufs=ld_bufs) as ld,
                        tc.tile_pool(name=f"pp_{_rep}", bufs=pp_bufs) as pp,
                        tc.tile_pool(name=f"sc_{_rep}", bufs=1) as sc,
                        tc.tile_pool(name=f"rb_{_rep}", bufs=3) as rbp,
                        tc.tile_pool(name=f"ot_{_rep}", bufs=2) as otp,
                        tc.tile_pool(name=f"p3y_{_rep}", bufs=4) as p3y,
                        tc.tile_pool(name=f"ps2s_{_rep}", bufs=2, space="PSUM") as ps2s,
                        tc.tile_pool(name=f"ps2o_{_rep}", bufs=o_bufs, space="PSUM") as ps2o,
                        tc.tile_pool(name=f"ps2m_{_rep}", bufs=m_bufs, space="PSUM") as ps2m,
                        tc.tile_pool(name=f"ps3_{_rep}", bufs=p3_bufs, space="PSUM") as ps3,
                    ):
                        rs_all = sc.tile([128, 512], F32, name=f"rsall_{_rep}")
                        rs_all_bf = sc.tile([128, 512], BF16, name=f"rsbf_{_rep}")
                        if 3 in phases:
                            nc.sync.dma_start(wp_all, wproj.ap())

                        pending = []
                        ot_chunks = {}

                        def do_p3(count):
                            for _ in range(count):
                                if not pending:
                                    return
                                pc, i = pending.pop(0)
                                ps_y = ps3.tile([128, 512], F32, tag="y")
                                for hh in range(HPC):
                                    nc.tensor.matmul(
                                        ps_y,
                                        lhsT=wp_all[:, i, hh, :],
                                        rhs=ot_chunks[pc][hh][:],
                                        start=(hh == 0),
                                        stop=(hh == HPC - 1),
                                    )
                                ys = p3y.tile([128, 512], F32, tag="ys")
                                nc.vector.tensor_copy(ys, ps_y)
                                nc.sync.dma_start(
                                    yT_r[:, i, 512 * pc : 512 * (pc + 1)], ys
                                )

                        for c in range(NT):
                            span = slice(512 * c, 512 * (c + 1))
                            ot_c = {}
                            pend_ot = []
                            for h in range(HPC):
                                qch = ld.tile([128, 512], BF16, tag="q")
                                nc.sync.dma_start(qch, qkvT[h][:, span])
                                if c == 0:
                                    nc.sync.dma_start(k_res[h], qkvT[HPC + h][:])

                                nblk = 4 * c + 4
                                pta = pp.tile(
                                    [128, 8, 512], BF16, tag="pTa", name=f"pta_{h}_{c}"
                                )
                                ptb = (
                                    pp.tile(
                                        [128, 8, 512], BF16, tag="pTb",
                                        name=f"ptb_{h}_{c}",
                                    )
                                    if nblk > 8
                                    else None
                                )

                                def pT(j):
                                    return (pta if j < 8 else ptb)[:, j % 8, :]

                                j = 0
                                while j < nblk:
                                    v = j - 4 * c
                                    if v < 0 and j + 1 < 4 * c:
                                        # full off-diagonal pair: one 2-bank
                                        # psum tile, one exp instruction
                                        ps_p = ps2s.tile([128, 2, 512], F32, tag="s")
                                        for u in (0, 1):
                                            nc.tensor.matmul(
                                                ps_p[:, u, :],
                                                lhsT=k_res[h][
                                                    :, 128 * (j + u) : 128 * (j + u + 1)
                                                ],
                                                rhs=qch,
                                                start=True,
                                                stop=True,
                                            )
                                        jj = j % 8
                                        pt_t = pta if j < 8 else ptb
                                        if probe == "halfexp":
                                            nc.scalar.activation(
                                                pt_t[:, jj : jj + 2, 0:256],
                                                ps_p[:, :, 0:256], AF.Exp,
                                                scale=SCALE,
                                            )
                                        else:
                                            nc.scalar.activation(
                                                pt_t[:, jj : jj + 2, :], ps_p, AF.Exp,
                                                scale=SCALE,
                                            )
                                        j += 2
                                        continue
                                    if diag_pair and v >= 0 and v % 2 == 0 and j + 1 < nblk and (j % 8) < 7:
                                        # two diagonal-group blocks, one exp over
                                        # both full slots (stale cols unread)
                                        ps_p = ps2s.tile([128, 2, 512], F32, tag="s")
                                        for u in (0, 1):
                                            offu = 128 * (v + u) if v + u > 0 else 0
                                            nc.tensor.matmul(
                                                ps_p[:, u, offu:512],
                                                lhsT=k_res[h][
                                                    :, 128 * (j + u) : 128 * (j + u + 1)
                                                ],
                                                rhs=qch[:, offu:512],
                                                start=True,
                                                stop=True,
                                            )
                                        jj = j % 8
                                        pt_t = pta if j < 8 else ptb
                                        nc.scalar.activation(
                                            pt_t[:, jj : jj + 2, :], ps_p, AF.Exp,
                                            scale=SCALE,
                                        )
                                        for u in (0, 1):
                                            offu = 128 * (v + u)
                                            if v + u >= 0:
                                                nc.vector.tensor_mul(
                                                    pT(j + u)[:, offu : offu + 128],
                                                    pT(j + u)[:, offu : offu + 128],
                                                    tri_sb,
                                                )
                                        j += 2
                                        continue
                                    off = 128 * v if v > 0 else 0
                                    ps_p = ps2s.tile([128, 2, 512], F32, tag="s")
                                    nc.tensor.matmul(
                                        ps_p[:, 0, off:512],
                                        lhsT=k_res[h][:, 128 * j : 128 * (j + 1)],
                                        rhs=qch[:, off:512],
                                        start=True,
                                        stop=True,
                                    )
                                    nc.scalar.activation(
                                        pT(j)[:, off:512], ps_p[:, 0, off:512], AF.Exp,
                                        scale=SCALE,
                                    )
                                    if v >= 0 and probe != "notri":
                                        nc.vector.tensor_mul(
                                            pT(j)[:, off : off + 128],
                                            pT(j)[:, off : off + 128],
                                            tri_sb,
                                        )
                                    j += 1

                                # out^T = V P^T
                                ps_o = ps2o.tile([128, 512], F32, tag="o")
                                for j in range(nblk):
                                    v = j - 4 * c
                                    off = 128 * v if v > 0 else 0
                                    nc.tensor.matmul(
                                        ps_o[:, off:512],
                                        lhsT=v_sb[h][:, 128 * j : 128 * (j + 1)],
                                        rhs=pT(j)[:, off:512],
                                        start=(j == 0),
                                        stop=(j == nblk - 1),
                                    )

                                # colsums on PE into a shared psum bank
                                if probe == "nosum":
                                    rb = rbp.tile([128, 512], F32, tag="rb")
                                    nc.vector.memset(rb, 0.01)
                                    ot = otp.tile(
                                        [128, 512], BF16, tag=f"ot{h}",
                                        name=f"ot_{h}_{c}",
                                    )
                                    nc.vector.tensor_tensor(ot, ps_o, rb, MUL)
                                    ot_c[h] = ot
                                    if 3 in phases:
                                        do_p3(2)
                                    continue
                                base = 0
                                sum_t = ps2m.tile([128, 512], F32, tag="sum")
                                ssl = sum_t[base : base + 1, :]
                                if probe == "nochain":
                                    for j in range(nblk):
                                        v = j - 4 * c
                                        off = 128 * v if v > 0 else 0
                                        nc.tensor.matmul(
                                            ssl[:, off:512],
                                            lhsT=ones_bf,
                                            rhs=pT(j)[:, off:512],
                                            start=(j == 0),
                                            stop=(j == nblk - 1),
                                        )
                                    rb = rbp.tile([128, 512], F32, tag="rb")
                                    nc.vector.memset(rb, 0.01)
                                    ot = otp.tile(
                                        [128, 512], BF16, tag=f"ot{h}",
                                        name=f"ot_{h}_{c}",
                                    )
                                    nc.vector.tensor_tensor(ot, ps_o, rb, MUL)
                                    ot_c[h] = ot
                                    if 3 in phases:
                                        do_p3(2)
                                    continue
                                for j in range(nblk):
                                    v = j - 4 * c
                                    off = 128 * v if v > 0 else 0
                                    nc.tensor.matmul(
                                        ssl[:, off:512],
                                        lhsT=ones_bf,
                                        rhs=pT(j)[:, off:512],
                                        start=(j == 0),
                                        stop=(j == nblk - 1),
                                    )
                                rsl = rs_all[base : base + 1, :]
                                if bcast == "pe":
                                    # reciprocal in bf16, broadcast across
                                    # partitions via a 1-row PE outer product
                                    # into the (already-read) sum bank
                                    rs_bf = rs_all_bf[base : base + 1, :]
                                    with nc.allow_low_precision(
                                        reason="bf16 softmax reciprocal: ~0.4% "
                                        "common-mode scale error, well inside "
                                        "the 2e-2 budget"
                                    ):
                                        nc.vector.reciprocal(rs_bf, ssl)
                                    rb = ps2m.tile([128, 512], F32, tag="sum")
                                    nc.tensor.matmul(
                                        rb, lhsT=ones_row, rhs=rs_bf,
                                        start=True, stop=True,
                                    )
                                    ot = otp.tile(
                                        [128, 512], BF16, tag=f"ot{h}",
                                        name=f"ot_{h}_{c}",
                                    )
                                    nc.vector.tensor_tensor(ot, ps_o, rb, MUL)
                                    ot_c[h] = ot
                                    if 3 in phases:
                                        do_p3(2)
                                    continue
                                nc.vector.reciprocal(rsl, ssl)
                                rb = rbp.tile([128, 512], F32, tag="rb")
                                if bcast == "pool":
                                    nc.gpsimd.partition_broadcast(rb, rsl)
                                else:
                                    rbx = dram.tile(
                                        [1, 512], F32, tag=f"rbx{h % 3}",
                                        name=f"rbx_{h % 3}",
                                    )
                                    nc.sync.dma_start(rbx, rsl)
                                    nc.gpsimd.dma_start(
                                        rb, rbx[0].partition_broadcast(128)
                                    )
                                # delay the normalize one slot so the Pool
                                # broadcast never blocks the DVE queue
                                pend_ot.append((h, ps_o, rb))
                                if len(pend_ot) > ot_lag:
                                    ph, p_pso, p_rb = pend_ot.pop(0)
                                    ot = otp.tile(
                                        [128, 512], BF16, tag=f"ot{ph}",
                                        name=f"ot_{ph}_{c}",
                                    )
                                    nc.vector.tensor_tensor(ot, p_pso, p_rb, MUL)
                                    ot_c[ph] = ot
                                    if 3 in phases:
                                        do_p3(2)

                            while pend_ot:
                                ph, p_pso, p_rb = pend_ot.pop(0)
                                ot = otp.tile(
                                    [128, 512], BF16, tag=f"ot{ph}",
                                    name=f"ot_{ph}_{c}",
                                )
                                nc.vector.tensor_tensor(ot, p_pso, p_rb, MUL)
                                ot_c[ph] = ot
                                if 3 in phases:
                                    do_p3(2)

                            ot_chunks[c] = ot_c
                            if 3 in phases:
                                pending.extend((c, i) for i in range(CT))
                        if 3 in phases:
                            do_p3(len(pending))
                kvp_ctx.__exit__(None, None, None)

    nc.compile()
    return nc


USE_V3 = True
_BUILD_V2 = build_nc


def build_nc(phases=(1, 2, 3), reps=1, **kw):
    if USE_V3:
        return build_nc_v3(phases=phases, reps=reps, **kw)
    return _BUILD_V2(phases=phases, reps=reps)


_NC_CACHE = None


def _get_nc():
    global _NC_CACHE
    if _NC_CACHE is None:
        _NC_CACHE = build_nc()
    return _NC_CACHE


def _prep_inputs(x, Wqkv, bqkv, Wproj):
    """Host-side shard + pre-tile + bf16 conversion. Returns list of 8 in_maps,
    core index = g * B + b."""
    bf16 = mybir.dt.np(BF16)
    x = np.asarray(x, dtype=np.float32)
    Wqkv = np.asarray(Wqkv, dtype=np.float32)
    Wproj = np.asarray(Wproj, dtype=np.float32)
    bqkv = np.asarray(bqkv, dtype=np.float32)

    # tri[p, f] = 1 if f >= p else 0 (keep keys p <= query f on diag blocks)
    p = np.arange(128)[:, None]
    f = np.arange(128)[None, :]
    tri = np.ascontiguousarray((f >= p).astype(bf16))

    # xT tiles per batch: [128, CT, T] with [p, o, t] = x[b, t, o*128+p]
    xT_b = []
    for b in range(B):
        xt = x[b].T  # [C, T]
        xT_b.append(
            np.ascontiguousarray(
                xt.reshape(CT, 128, T).transpose(1, 0, 2).astype(bf16)
            )
        )

    in_maps = [None] * (G * B)
    for g in range(G):
        cols = np.concatenate(
            [
                np.arange(g * 1024, (g + 1) * 1024),
                np.arange(C + g * 1024, C + (g + 1) * 1024),
                np.arange(2 * C + g * 1024, 2 * C + (g + 1) * 1024),
            ]
        )
        wg = Wqkv[:, cols]  # [C, 3072] = [(o p), (j m)]
        # -> [128 p, 24 j, 16 o, 128 m]
        wg_t = np.ascontiguousarray(
            wg.reshape(CT, 128, NJ, 128).transpose(1, 2, 0, 3).astype(bf16)
        )
        bg = bqkv[cols]  # [3072]
        bg_t = np.ascontiguousarray(bg.reshape(NJ, 128).T)  # [128, 24]
        wp = Wproj[g * 1024 : (g + 1) * 1024, :]  # [1024, C] = [(h p), (i m)]
        # -> [128 p, 16 i, 8 h, 128 m]
        wp_t = np.ascontiguousarray(
            wp.reshape(HPC, 128, CT, 128).transpose(1, 2, 0, 3).astype(bf16)
        )
        for b in range(B):
            in_maps[g * B + b] = dict(
                xT=xT_b[b], wqkv=wg_t, wproj=wp_t, bqkv=bg_t, tri=tri
            )
    return in_maps


def kernel(x, Wqkv, bqkv, Wproj, bproj):
    x = np.asarray(x)
    nc = _get_nc()
    in_maps = _prep_inputs(x, Wqkv, bqkv, Wproj)
    res = run_bass_kernel_spmd(nc, in_maps, core_ids=list(range(G * B)))
    y = np.empty((B, T, C), dtype=np.float32)
    bp = np.asarray(bproj, dtype=np.float32)
    if USE_V3:
        # v3 computes V without its qkv bias; since softmax rows sum to 1,
        # the v-bias contributes exactly bqkv_v @ Wproj to every output row.
        bp = bp + np.asarray(bqkv, dtype=np.float32)[2 * C :] @ np.asarray(
            Wproj, dtype=np.float32
        )
    for b in range(B):
        acc = res.results[b]["yT"].astype(np.float32).copy()
        for g in range(1, G):
            acc += res.results[g * B + b]["yT"]
        y[b] = acc.T + bp[None, :]
    return y



# revision 28
# speedup vs baseline: 1.0321x; 1.0321x over previous
"""Causal self-attention block (B=4, T=2048, C=2048, H=16, D=128) on 8 trn2 cores.

Sharding: tensor-parallel over head groups (2 groups of 8 heads) x
data-parallel over batch (4). Core (g, b) computes, for batch b and heads
[8g, 8g+8): qkv projection, causal attention, and the partial output
projection contribution attn_out[:, heads_g] @ Wproj[rows_g]. The host sums
the two partial yT per batch, adds bproj, and transposes back.

v3 (active, build_nc_v3): phase 2 runs c-outer/h-inner with phase 3
interleaved two output tiles per head-slot, so the PE never drains at a
phase boundary. V is produced in NATURAL [t,d] layout directly in phase 1
(128-wide matmuls at equal PE cost; v-bias folded into bproj on the host
since softmax rows sum to 1), which removes all per-slot PE transposes.
Softmax colsums are per-block ones-matmuls on the PE (no DVE chains);
reciprocal broadcast via Pool partition_broadcast (no DRAM round-trip;
source slice must sit at partition base 0). Off-diagonal S blocks are
computed in [128,2,512] psum pairs and exp'd with ONE Act instruction per
pair — Act per-instruction overhead, not exp payload, was the phase-2
binder. qkvT DRAM tiles are double-buffered by rep parity and v-tiles are
emitted last in phase 1, so rep r+1's q/k projection overlaps rep r's
attention (~-138us/rep in the slope bench). PSUM start=True resets the
whole bank: never interleave accumulation groups in one bank.

v2 (kept as _BUILD_V2): h-outer phases, DVE colsum chains, DRAM-round-trip
reciprocal broadcast, PE transposes of V^T per chunk.
"""

import sys

sys.path.insert(0, "/opt/trn_rl_repo")

import numpy as np

import concourse.bass as bass
import concourse.mybir as mybir
import concourse.tile as tile
from concourse import bacc
from concourse.bass_utils import run_bass_kernel_spmd
from concourse.masks import make_identity

F32 = mybir.dt.float32
F32R = mybir.dt.float32r
BF16 = mybir.dt.bfloat16
AF = mybir.ActivationFunctionType
MUL = mybir.AluOpType.mult

B, T, C = 4, 2048, 2048
H, D = 16, 128
G = 2  # head-group shards
HPC = H // G  # heads per core = 8
CT = C // 128  # contraction chunks = 16
NT = T // 512  # t chunks of 512 = 4
NJ = 3 * HPC  # qkv col tiles per core = 24
SCALE = 1.0 / float(np.sqrt(D))
# off-diagonal blocks per chunk handed to the Pool accumulation chain
POOL_BLOCKS = {0: 0, 1: 2, 2: 4, 3: 6}
SUM_CHAINS = "dve"  # 'dve_pool' | 'dve' (no Pool adds) | 'pe' (baseline)
BCAST = "dram"  # 'pool' (partition_broadcast op) | 'dram' (DMA round-trip)
QKV_SBUF = False  # keep qkvT in SBUF (no DRAM round-trip between phases 1/2)


def build_nc(phases=(1, 2, 3), reps=1):
    nc = bacc.Bacc("TRN2", target_bir_lowering=False)
    xT = nc.dram_tensor("xT", [128, CT, T], BF16, kind="ExternalInput")
    wqkv = nc.dram_tensor("wqkv", [128, NJ, CT, 128], BF16, kind="ExternalInput")
    wproj = nc.dram_tensor("wproj", [128, CT, HPC, 128], BF16, kind="ExternalInput")
    bqkv = nc.dram_tensor("bqkv", [128, NJ], F32, kind="ExternalInput")
    tri_in = nc.dram_tensor("tri", [128, 128], BF16, kind="ExternalInput")
    yT = nc.dram_tensor("yT", [C, T], F32, kind="ExternalOutput")
    yT_r = yT.rearrange("(i p) t -> p i t", p=128)

    with tile.TileContext(nc) as tc:
        with (
            tc.tile_pool(name="const", bufs=1) as cst,
            tc.tile_pool(name="dram", bufs=1, space="DRAM") as dram,
        ):
            # allocate constants up front; their loads are emitted after the
            # warm-start DMAs so the first matmul's deps go first in the queue
            tri_sb = cst.tile([128, 128], BF16)
            bias_sb = cst.tile([128, NJ], F32)
            ident = cst.tile([128, 128], BF16)
            ones_f = cst.tile([128, 1], F32)
            ones = cst.tile([128, 1], F32R)
            ones_bf = cst.tile([128, 1], BF16)

            def load_consts():
                nc.sync.dma_start(tri_sb, tri_in.ap())
                nc.sync.dma_start(bias_sb, bqkv.ap())
                make_identity(nc, ident)
                nc.vector.memset(ones_f, 1.0)
                nc.vector.tensor_copy(ones, ones_f)
                nc.vector.tensor_copy(ones_bf, ones_f)

            if not QKV_SBUF:
                qkvT = [
                    dram.tile([128, T], BF16, name=f"qkvT{j}", tag=f"qkvT{j}")
                    for j in range(NJ)
                ]

            if 1 not in phases:
                load_consts()

            from contextlib import ExitStack

            for _rep in range(reps):
              with ExitStack() as rep_ctx:
                  if QKV_SBUF:
                    qkvp = rep_ctx.enter_context(
                        tc.tile_pool(name=f"qkv_{_rep}", bufs=1)
                      )
                    qkvT = [
                        qkvp.tile([128, T], BF16, name=f"qkvsb{j}_{_rep}")
                          for j in range(NJ)
                      ]
                  # ---------------- phase 1: qkvT[col, t] = W^T x^T (+bias) -------
                  if 1 in phases:
                   with (
                      tc.tile_pool(name=f"p1x_{_rep}", bufs=1) as p1x,
                      tc.tile_pool(name=f"p1w_{_rep}", bufs=3) as p1w,
                      tc.tile_pool(name=f"p1s_{_rep}", bufs=4) as p1s,
                      tc.tile_pool(name=f"ps1_{_rep}", bufs=8, space="PSUM") as ps1,
                  ):
                      # interleave q/k/v col-tiles so head h's three tensors are all
                      # ready after 3*(h+1) of the 24 tiles
                      j_order = [base + h for h in range(HPC) for base in (0, HPC, 2 * HPC)]
                      WARM = 2  # first j's run chunk-outer to overlap the xs load
                      warm_w = {}
                      for wj in j_order[:WARM]:
                          w_sb = p1w.tile([128, CT, 128], BF16, tag="w")
                          nc.sync.dma_start(w_sb, wqkv[:, wj])
                          warm_w[wj] = w_sb
                      xs = p1x.tile([128, CT, T], BF16)
                      for cc in range(CT):
                          nc.sync.dma_start(xs[:, cc, :], xT[:, cc, :])
                      if _rep == 0:
                          load_consts()
                      # warm-up: 8 psum groups accumulate chunk-by-chunk as the xs
                      # chunks arrive, so PE works during the x load
                      warm_ps = {
                          (wj, c): ps1.tile(
                              [128, 512], F32, tag="ps", name=f"warm_ps_{wj}_{c}"
                          )
                          for wj in j_order[:WARM]
                          for c in range(NT)
                      }
                      for cc in range(CT):
                          for wj in j_order[:WARM]:
                              for c in range(NT):
                                  nc.tensor.matmul(
                                      warm_ps[(wj, c)],
                                      lhsT=warm_w[wj][:, cc, :],
                                      rhs=xs[:, cc, 512 * c : 512 * (c + 1)],
                                      start=(cc == 0),
                                      stop=(cc == CT - 1),
                                  )
                      for wj in j_order[:WARM]:
                          for c in range(NT):
                              if QKV_SBUF:
                                  nc.vector.tensor_scalar_add(
                                      qkvT[wj][:, 512 * c : 512 * (c + 1)],
                                      warm_ps[(wj, c)],
                                      bias_sb[:, wj : wj + 1],
                                  )
                              else:
                                  st = p1s.tile([128, 512], BF16, tag="st")
                                  nc.vector.tensor_scalar_add(
                                      st, warm_ps[(wj, c)], bias_sb[:, wj : wj + 1]
                                  )
                                  nc.sync.dma_start(
                                      qkvT[wj][:, 512 * c : 512 * (c + 1)], st
                                  )
                      for j in j_order[WARM:]:
                          w_sb = p1w.tile([128, CT, 128], BF16, tag="w")
                          nc.sync.dma_start(w_sb, wqkv[:, j])
                          for c in range(NT):
                              ps = ps1.tile([128, 512], F32, tag="ps")
                              for cc in range(CT):
                                  nc.tensor.matmul(
                                      ps,
                                      lhsT=w_sb[:, cc, :],
                                      rhs=xs[:, cc, 512 * c : 512 * (c + 1)],
                                      start=(cc == 0),
                                      stop=(cc == CT - 1),
                                  )
                              if QKV_SBUF:
                                  nc.vector.tensor_scalar_add(
                                      qkvT[j][:, 512 * c : 512 * (c + 1)],
                                      ps,
                                      bias_sb[:, j : j + 1],
                                  )
                              else:
                                  st = p1s.tile([128, 512], BF16, tag="st")
                                  nc.vector.tensor_scalar_add(
                                      st, ps, bias_sb[:, j : j + 1]
                                  )
                                  nc.sync.dma_start(
                                      qkvT[j][:, 512 * c : 512 * (c + 1)], st
                                  )

                  # phase 0: memset ot_tiles (phase-3 isolation benchmark mode)
                  if 0 in phases:
                   with tc.tile_pool(name=f"otp_{_rep}", bufs=1) as otp:
                    ot_tiles = {}
                    for h in range(HPC):
                        for c in range(NT):
                            ot = otp.tile(
                                [128, 512], BF16, name=f"ot_{h}_{c}", tag=f"ot_{h}_{c}"
                            )
                            nc.vector.memset(ot, 0.01)
                            ot_tiles[(h, c)] = ot
                    if 3 in phases:
                     with (
                        tc.tile_pool(name=f"p3w_{_rep}", bufs=3) as p3w,
                        tc.tile_pool(name=f"p3y_{_rep}", bufs=4) as p3y,
                        tc.tile_pool(name=f"ps3_{_rep}", bufs=4, space="PSUM") as ps3,
                     ):
                      for i in range(CT):
                          wp = p3w.tile([128, HPC, 128], BF16, tag="wp")
                          nc.sync.dma_start(wp, wproj[:, i])
                          for c in range(NT):
                              ps_y = ps3.tile([128, 512], F32, tag="y")
                              for hh in range(HPC):
                                  nc.tensor.matmul(
                                      ps_y,
                                      lhsT=wp[:, hh, :],
                                      rhs=ot_tiles[(hh, c)][:],
                                      start=(hh == 0),
                                      stop=(hh == HPC - 1),
                                  )
                              ys = p3y.tile([128, 512], F32, tag="ys")
                              nc.scalar.copy(ys, ps_y)
                              nc.sync.dma_start(yT_r[:, i, 512 * c : 512 * (c + 1)], ys)

                  # ---- phases 2+3 share a persistent SBUF pool holding the
                  # attention outputs (no DRAM round-trip, no phase-3 reload) ----
                  if 2 in phases:
                   with tc.tile_pool(name=f"otp_{_rep}", bufs=1) as otp:
                    ot_tiles = {}
                    with (
                      tc.tile_pool(name=f"p2qk_{_rep}", bufs=2) as p2qk,
                      tc.tile_pool(name=f"p2v_{_rep}", bufs=2) as p2v,
                      tc.tile_pool(name=f"p2p_{_rep}", bufs=3) as p2p,
                      tc.tile_pool(name=f"p2sc_{_rep}", bufs=4) as p2sc,
                      tc.tile_pool(name=f"ps2s_{_rep}", bufs=3, space="PSUM") as ps2s,
                      tc.tile_pool(name=f"ps2t_{_rep}", bufs=2, space="PSUM") as ps2t,
                      tc.tile_pool(name=f"ps2m_{_rep}", bufs=m_bufs, space="PSUM") as ps2m,
                      tc.tile_pool(name=f"ps2o_{_rep}", bufs=2, space="PSUM") as ps2o,
                      tc.tile_pool(name=f"dram_rb_{_rep}", bufs=4, space="DRAM") as dram_rb,
                    ):
                      for h in range(HPC):
                          if QKV_SBUF:
                              q_sb = qkvT[h]
                              k_sb = qkvT[HPC + h]
                              vt_sb = qkvT[2 * HPC + h]
                          else:
                              q_sb = p2qk.tile([128, T], BF16, tag="q")
                              nc.sync.dma_start(q_sb, qkvT[h][:])
                              k_sb = p2qk.tile([128, T], BF16, tag="k")
                              nc.sync.dma_start(k_sb, qkvT[HPC + h][:])
                              vt_sb = p2qk.tile([128, T], BF16, tag="vt")
                              nc.sync.dma_start(vt_sb, qkvT[2 * HPC + h][:])

                          # V natural layout via PE transposes of V^T blocks,
                          # spread per chunk (chunk c's AV needs blocks <= 4c+3)
                          v_sb = p2v.tile([128, T // 128, 128], BF16, tag="v")

                          tri = tri_sb
                          for c in range(NT):
                              for jb in range(4 * c, 4 * c + 4):
                                  ps_v = ps2t.tile([128, 128], BF16, tag="pst")
                                  nc.tensor.transpose(
                                      ps_v, vt_sb[:, 128 * jb : 128 * (jb + 1)], ident
                                  )
                                  nc.vector.tensor_copy(v_sb[:, jb, :], ps_v)

                              nblk = 4 * c + 4
                              pta = p2p.tile(
                                  [128, 8, 512], BF16, tag="pT", name=f"pta_{h}_{c}"
                              )
                              ptb = (
                                  p2p.tile(
                                      [128, 8, 512], BF16, tag="pT", name=f"ptb_{h}_{c}"
                                  )
                                  if nblk > 8
                                  else None
                              )

                              def pT(j):
                                  return (pta if j < 8 else ptb)[:, j % 8, :]

                              for j in range(nblk):
                                  v = j - 4 * c  # >= 0 on diagonal-group blocks
                                  off = 128 * v if v > 0 else 0
                                  ps_s = ps2s.tile([128, 512], F32, tag="s")
                                  nc.tensor.matmul(
                                      ps_s[:, off:512],
                                      lhsT=k_sb[:, 128 * j : 128 * (j + 1)],
                                      rhs=q_sb[:, 512 * c + off : 512 * (c + 1)],
                                      start=True,
                                      stop=True,
                                  )
                                  # exp only over the causally-reachable columns;
                                  # columns < off are never read downstream.
                                  nc.scalar.activation(
                                      pT(j)[:, off:512], ps_s[:, off:512], AF.Exp,
                                      scale=SCALE,
                                  )
                                  if v >= 0:
                                      nc.vector.tensor_mul(
                                          pT(j)[:, off : off + 128],
                                          pT(j)[:, off : off + 128],
                                          tri,
                                      )

                              # ---- softmax denominators ----
                              accA = accB = None
                              if SUM_CHAINS != "pe":
                                  # accumulation chains (DVE + optionally Pool),
                                  # finished by matmuls into one [1,512] group
                                  npool = POOL_BLOCKS[c] if SUM_CHAINS == "dve_pool" else 0
                                  b_list = list(range(npool))  # off-diag, full
                                  a_off = list(range(npool, 4 * c))  # off-diag rest
                                  accA = p2sc.tile(
                                      [128, 512], F32R, tag="acc", name=f"accA_{h}_{c}"
                                  )
                                  if a_off:
                                      # init with two full-width blocks
                                      nc.vector.tensor_add(accA, pT(4 * c), pT(a_off[0]))
                                      for j in a_off[1:]:
                                          nc.vector.tensor_add(accA, accA, pT(j))
                                  else:
                                      nc.vector.tensor_copy(accA, pT(4 * c))
                                  for v in range(1, 4):
                                      off = 128 * v
                                      nc.vector.tensor_add(
                                          accA[:, off:512],
                                          accA[:, off:512],
                                          pT(4 * c + v)[:, off:512],
                                      )
                                  if npool:
                                      accB = p2sc.tile(
                                          [128, 512], F32R, tag="acc",
                                          name=f"accB_{h}_{c}",
                                      )
                                      nc.gpsimd.tensor_add(
                                          accB, pT(b_list[0]), pT(b_list[1])
                                      )
                                      for j in b_list[2:]:
                                          nc.gpsimd.tensor_add(accB, accB, pT(j))

                              # ---- out^T = V P^T (before the ones-matmuls so
                              # the PE never waits on the DVE/Pool chains) ----
                              ps_o = ps2o.tile([128, 512], F32, tag="o")
                              for j in range(nblk):
                                  v = j - 4 * c
                                  off = 128 * v if v > 0 else 0
                                  nc.tensor.matmul(
                                      ps_o[:, off:512],
                                      lhsT=v_sb[:, j, :],
                                      rhs=pT(j)[:, off:512],
                                      start=(j == 0),
                                      stop=(j == nblk - 1),
                                  )

                              ps_sum = ps2m.tile([1, 512], F32, tag="sum")
                              if SUM_CHAINS == "pe":
                                  for j in range(nblk):
                                      v = j - 4 * c
                                      off = 128 * v if v > 0 else 0
                                      nc.tensor.matmul(
                                          ps_sum[:, off:512],
                                          lhsT=ones_bf,
                                          rhs=pT(j)[:, off:512],
                                          start=(j == 0),
                                          stop=(j == nblk - 1),
                                      )
                              else:
                                  nc.tensor.matmul(
                                      ps_sum, lhsT=ones, rhs=accA,
                                      start=True, stop=(accB is None),
                                  )
                                  if accB is not None:
                                      nc.tensor.matmul(
                                          ps_sum, lhsT=ones, rhs=accB,
                                          start=False, stop=True,
                                      )
                              rs = p2sc.tile([1, 512], F32, tag="rs")
                              nc.vector.reciprocal(rs, ps_sum[0:1, :])
                              rb = p2sc.tile([128, 512], F32, tag="rb")
                              if BCAST == "pool":
                                  nc.gpsimd.partition_broadcast(rb, rs)
                              else:
                                  rbx = dram_rb.tile([1, 512], F32, tag="rbx")
                                  nc.sync.dma_start(rbx, rs)
                                  nc.gpsimd.dma_start(
                                      rb, rbx[0].partition_broadcast(128)
                                  )

                              ot = otp.tile(
                                  [128, 512], BF16, name=f"ot_{h}_{c}", tag=f"ot_{h}_{c}"
                              )
                              nc.vector.tensor_tensor(ot, ps_o, rb, MUL)
                              ot_tiles[(h, c)] = ot

                    # -------- phase 3: yT = Wproj_g^T attn_outT (from SBUF) -------
                    if 3 in phases:
                     with (
                        tc.tile_pool(name=f"p3w_{_rep}", bufs=3) as p3w,
                        tc.tile_pool(name=f"p3y_{_rep}", bufs=4) as p3y,
                        tc.tile_pool(name=f"ps3_{_rep}", bufs=4, space="PSUM") as ps3,
                     ):
                      for i in range(CT):
                          wp = p3w.tile([128, HPC, 128], BF16, tag="wp")
                          nc.sync.dma_start(wp, wproj[:, i])
                          for c in range(NT):
                              ps_y = ps3.tile([128, 512], F32, tag="y")
                              for hh in range(HPC):
                                  nc.tensor.matmul(
                                      ps_y,
                                      lhsT=wp[:, hh, :],
                                      rhs=ot_tiles[(hh, c)][:],
                                      start=(hh == 0),
                                      stop=(hh == HPC - 1),
                                  )
                              ys = p3y.tile([128, 512], F32, tag="ys")
                              nc.scalar.copy(ys, ps_y)
                              nc.sync.dma_start(yT_r[:, i, 512 * c : 512 * (c + 1)], ys)

    nc.compile()
    return nc


def build_nc_v3(phases=(1, 2, 3), reps=1, bcast="dram", pp_bufs=2, ld_bufs=3, diag_pair=False, probe=None, o_bufs=2, m_bufs=1, p3_bufs=1, ot_lag=1):
    """c-outer/h-inner phases 2+3, interleaved per chunk.

    vs v2: softmax colsums via per-block ones-matmuls on the PE (no DVE
    accumulation chains), reciprocal broadcast on Pool (no DRAM round-trip),
    Act engine runs ONLY Exp (no activation-table swaps), phase 3 runs per
    chunk right after the 8 heads' ot tiles for that chunk are ready (PE
    keeps streaming instead of draining at the phase boundary), phase-3
    psum->sbuf copies moved to DVE.
    """
    nc = bacc.Bacc("TRN2", target_bir_lowering=False)
    xT = nc.dram_tensor("xT", [128, CT, T], BF16, kind="ExternalInput")
    wqkv = nc.dram_tensor("wqkv", [128, NJ, CT, 128], BF16, kind="ExternalInput")
    wproj = nc.dram_tensor("wproj", [128, CT, HPC, 128], BF16, kind="ExternalInput")
    bqkv = nc.dram_tensor("bqkv", [128, NJ], F32, kind="ExternalInput")
    tri_in = nc.dram_tensor("tri", [128, 128], BF16, kind="ExternalInput")
    yT = nc.dram_tensor("yT", [C, T], F32, kind="ExternalOutput")
    yT_r = yT.rearrange("(i p) t -> p i t", p=128)

    with tile.TileContext(nc) as tc:
        with (
            tc.tile_pool(name="const", bufs=1) as cst,
            tc.tile_pool(name="dram", bufs=1, space="DRAM") as dram,
        ):
            tri_sb = cst.tile([128, 128], BF16)
            bias_sb = cst.tile([128, NJ], F32)
            ident = cst.tile([128, 128], BF16)
            ones_f = cst.tile([128, 1], F32)
            ones_bf = cst.tile([128, 1], BF16)
            ones_row = cst.tile([1, 128], BF16)

            def load_consts():
                nc.vector.memset(ones_row, 1.0)
                nc.sync.dma_start(tri_sb, tri_in.ap())
                nc.sync.dma_start(bias_sb, bqkv.ap())
                make_identity(nc, ident)
                nc.vector.memset(ones_f, 1.0)
                nc.vector.tensor_copy(ones_bf, ones_f)

            qkvT_db = [
                [
                    dram.tile([128, T], BF16, name=f"qkvT{j}_{p}", tag=f"qkvT{j}_{p}")
                    for j in range(NJ)
                ]
                for p in range(2)
            ]

            if 1 not in phases:
                load_consts()

            for _rep in range(reps):
                qkvT = qkvT_db[_rep % 2]
                # persistent across phases: K residents, V natural, Wproj
                kvp_ctx = tc.tile_pool(name=f"kv_{_rep}", bufs=1)
                kvp = kvp_ctx.__enter__()
                k_res = {
                    h: kvp.tile([128, T], BF16, name=f"kres_{h}_{_rep}")
                    for h in range(HPC)
                }
                v_sb = {
                    h: kvp.tile([128, T], BF16, name=f"vsb_{h}_{_rep}")
                    for h in range(HPC)
                }
                wp_all = kvp.tile([128, CT, HPC, 128], BF16, name=f"wpall_{_rep}")

                # ---------------- phase 1: qkvT[col, t] = W^T x^T (+bias) -----
                if 1 in phases:
                    with (
                        tc.tile_pool(name=f"p1x_{_rep}", bufs=1) as p1x,
                        tc.tile_pool(name=f"p1w_{_rep}", bufs=3) as p1w,
                        tc.tile_pool(name=f"p1s_{_rep}", bufs=4) as p1s,
                        tc.tile_pool(name=f"ps1_{_rep}", bufs=8, space="PSUM") as ps1,
                    ):
                        j_order = [
                            base + h for h in range(HPC) for base in (0, HPC)
                        ] + [2 * HPC + h for h in range(HPC)]
                        WARM = 2
                        warm_w = {}
                        for wj in j_order[:WARM]:
                            w_sb = p1w.tile([128, CT, 128], BF16, tag="w")
                            nc.sync.dma_start(w_sb, wqkv[:, wj])
                            warm_w[wj] = w_sb
                        xs = p1x.tile([128, CT, T], BF16)
                        for cc in range(CT):
                            nc.sync.dma_start(xs[:, cc, :], xT[:, cc, :])
                        if _rep == 0:
                            load_consts()
                        warm_ps = {
                            (wj, c): ps1.tile(
                                [128, 512], F32, tag="ps", name=f"warm_ps_{wj}_{c}"
                            )
                            for wj in j_order[:WARM]
                            for c in range(NT)
                        }
                        for cc in range(CT):
                            for wj in j_order[:WARM]:
                                for c in range(NT):
                                    nc.tensor.matmul(
                                        warm_ps[(wj, c)],
                                        lhsT=warm_w[wj][:, cc, :],
                                        rhs=xs[:, cc, 512 * c : 512 * (c + 1)],
                                        start=(cc == 0),
                                        stop=(cc == CT - 1),
                                    )
                        for wj in j_order[:WARM]:
                            for c in range(NT):
                                st = p1s.tile([128, 512], BF16, tag="st")
                                nc.vector.tensor_scalar_add(
                                    st, warm_ps[(wj, c)], bias_sb[:, wj : wj + 1]
                                )
                                nc.sync.dma_start(
                                    qkvT[wj][:, 512 * c : 512 * (c + 1)], st
                                )
                        for j in j_order[WARM:]:
                            w_sb = p1w.tile([128, CT, 128], BF16, tag="w")
                            nc.sync.dma_start(w_sb, wqkv[:, j])
                            if j >= 2 * HPC:
                                # V in natural layout [t, d], written straight
                                # into the persistent v_sb (bias folded into
                                # bproj on the host; softmax rows sum to 1)
                                vh = j - 2 * HPC
                                for tb in range(CT):
                                    ps = ps1.tile([128, 512], F32, tag="ps")
                                    for cc in range(CT):
                                        nc.tensor.matmul(
                                            ps[:, 0:128],
                                            lhsT=xs[:, cc, 128 * tb : 128 * (tb + 1)],
                                            rhs=w_sb[:, cc, :],
                                            start=(cc == 0),
                                            stop=(cc == CT - 1),
                                        )
                                    nc.vector.tensor_copy(
                                        v_sb[vh][:, 128 * tb : 128 * (tb + 1)],
                                        ps[:, 0:128],
                                    )
                                continue
                            for c in range(NT):
                                ps = ps1.tile([128, 512], F32, tag="ps")
                                for cc in range(CT):
                                    nc.tensor.matmul(
                                        ps,
                                        lhsT=w_sb[:, cc, :],
                                        rhs=xs[:, cc, 512 * c : 512 * (c + 1)],
                                        start=(cc == 0),
                                        stop=(cc == CT - 1),
                                    )
                                st = p1s.tile([128, 512], BF16, tag="st")
                                nc.vector.tensor_scalar_add(
                                    st, ps, bias_sb[:, j : j + 1]
                                )
                                nc.sync.dma_start(
                                    qkvT[j][:, 512 * c : 512 * (c + 1)], st
                                )

                # ------------- phases 2+3: c-outer, phase 3 per chunk --------
                if 2 in phases:
                    with (
                        tc.tile_pool(name=f"ld_{_rep}", bufs=ld_bufs) as ld,
                        tc.tile_pool(name=f"pp_{_rep}", bufs=pp_bufs) as pp,
                        tc.tile_pool(name=f"sc_{_rep}", bufs=1) as sc,
                        tc.tile_pool(name=f"rb_{_rep}", bufs=3) as rbp,
                        tc.tile_pool(name=f"ot_{_rep}", bufs=2) as otp,
                        tc.tile_pool(name=f"p3y_{_rep}", bufs=4) as p3y,
                        tc.tile_pool(name=f"ps2s_{_rep}", bufs=2, space="PSUM") as ps2s,
                        tc.tile_pool(name=f"ps2o_{_rep}", bufs=o_bufs, space="PSUM") as ps2o,
                        tc.tile_pool(name=f"ps2m_{_rep}", bufs=m_bufs, space="PSUM") as ps2m,
                        tc.tile_pool(name=f"ps3_{_rep}", bufs=p3_bufs, space="PSUM") as ps3,
                    ):
                        rs_all = sc.tile([128, 512], F32, name=f"rsall_{_rep}")
                        rs_all_bf = sc.tile([128, 512], BF16, name=f"rsbf_{_rep}")
                        if 3 in phases:
                            nc.sync.dma_start(wp_all, wproj.ap())

                        pending = []
                        ot_chunks = {}

                        def do_p3(count):
                            for _ in range(count):
                                if not pending:
                                    return
                                pc, i = pending.pop(0)
                                ps_y = ps3.tile([128, 512], F32, tag="y")
                                for hh in range(HPC):
                                    nc.tensor.matmul(
                                        ps_y,
                                        lhsT=wp_all[:, i, hh, :],
                                        rhs=ot_chunks[pc][hh][:],
                                        start=(hh == 0),
                                        stop=(hh == HPC - 1),
                                    )
                                if probe != "noys":
                                    ys = p3y.tile([128, 512], F32, tag="ys")
                                    nc.vector.tensor_copy(ys, ps_y)
                                    nc.sync.dma_start(
                                        yT_r[:, i, 512 * pc : 512 * (pc + 1)], ys
                                    )

                        for c in range(NT):
                            span = slice(512 * c, 512 * (c + 1))
                            ot_c = {}
                            pend_ot = []
                            for h in range(HPC):
                                qch = ld.tile([128, 512], BF16, tag="q")
                                nc.sync.dma_start(qch, qkvT[h][:, span])
                                if c == 0:
                                    nc.sync.dma_start(k_res[h], qkvT[HPC + h][:])

                                nblk = 4 * c + 4
                                pta = pp.tile(
                                    [128, 8, 512], BF16, tag="pTa", name=f"pta_{h}_{c}"
                                )
                                ptb = (
                                    pp.tile(
                                        [128, 8, 512], BF16, tag="pTb",
                                        name=f"ptb_{h}_{c}",
                                    )
                                    if nblk > 8
                                    else None
                                )

                                def pT(j):
                                    return (pta if j < 8 else ptb)[:, j % 8, :]

                                j = 0
                                while j < nblk:
                                    v = j - 4 * c
                                    if v < 0 and j + 1 < 4 * c:
                                        # full off-diagonal pair: one 2-bank
                                        # psum tile, one exp instruction
                                        ps_p = ps2s.tile([128, 2, 512], F32, tag="s")
                                        for u in (0, 1):
                                            nc.tensor.matmul(
                                                ps_p[:, u, :],
                                                lhsT=k_res[h][
                                                    :, 128 * (j + u) : 128 * (j + u + 1)
                                                ],
                                                rhs=qch,
                                                start=True,
                                                stop=True,
                                            )
                                        jj = j % 8
                                        pt_t = pta if j < 8 else ptb
                                        if probe == "halfexp":
                                            nc.scalar.activation(
                                                pt_t[:, jj : jj + 2, 0:256],
                                                ps_p[:, :, 0:256], AF.Exp,
                                                scale=SCALE,
                                            )
                                        else:
                                            nc.scalar.activation(
                                                pt_t[:, jj : jj + 2, :], ps_p, AF.Exp,
                                                scale=SCALE,
                                            )
                                        j += 2
                                        continue
                                    if diag_pair and v >= 0 and v % 2 == 0 and j + 1 < nblk and (j % 8) < 7:
                                        # two diagonal-group blocks, one exp over
                                        # both full slots (stale cols unread)
                                        ps_p = ps2s.tile([128, 2, 512], F32, tag="s")
                                        for u in (0, 1):
                                            offu = 128 * (v + u) if v + u > 0 else 0
                                            nc.tensor.matmul(
                                                ps_p[:, u, offu:512],
                                                lhsT=k_res[h][
                                                    :, 128 * (j + u) : 128 * (j + u + 1)
                                                ],
                                                rhs=qch[:, offu:512],
                                                start=True,
                                                stop=True,
                                            )
                                        jj = j % 8
                                        pt_t = pta if j < 8 else ptb
                                        nc.scalar.activation(
                                            pt_t[:, jj : jj + 2, :], ps_p, AF.Exp,
                                            scale=SCALE,
                                        )
                                        for u in (0, 1):
                                            offu = 128 * (v + u)
                                            if v + u >= 0:
                                                nc.vector.tensor_mul(
                                                    pT(j + u)[:, offu : offu + 128],
                                                    pT(j + u)[:, offu : offu + 128],
                                                    tri_sb,
                                                )
                                        j += 2
                                        continue
                                    off = 128 * v if v > 0 else 0
                                    ps_p = ps2s.tile([128, 2, 512], F32, tag="s")
                                    nc.tensor.matmul(
                                        ps_p[:, 0, off:512],
                                        lhsT=k_res[h][:, 128 * j : 128 * (j + 1)],
                                        rhs=qch[:, off:512],
                                        start=True,
                                        stop=True,
                                    )
                                    nc.scalar.activation(
                                        pT(j)[:, off:512], ps_p[:, 0, off:512], AF.Exp,
                                        scale=SCALE,
                                    )
                                    if v >= 0 and probe != "notri":
                                        nc.vector.tensor_mul(
                                            pT(j)[:, off : off + 128],
                                            pT(j)[:, off : off + 128],
                                            tri_sb,
                                        )
                                    j += 1

                                # out^T = V P^T
                                ps_o = ps2o.tile([128, 512], F32, tag="o")
                                for j in range(nblk):
                                    v = j - 4 * c
                                    off = 128 * v if v > 0 else 0
                                    nc.tensor.matmul(
                                        ps_o[:, off:512],
                                        lhsT=v_sb[h][:, 128 * j : 128 * (j + 1)],
                                        rhs=pT(j)[:, off:512],
                                        start=(j == 0),
                                        stop=(j == nblk - 1),
                                    )

                                # colsums on PE into a shared psum bank
                                if probe == "nosum":
                                    rb = rbp.tile([128, 512], F32, tag="rb")
                                    nc.vector.memset(rb, 0.01)
                                    ot = otp.tile(
                                        [128, 512], BF16, tag=f"ot{h}",
                                        name=f"ot_{h}_{c}",
                                    )
                                    nc.vector.tensor_tensor(ot, ps_o, rb, MUL)
                                    ot_c[h] = ot
                                    if 3 in phases:
                                        do_p3(2)
                                    continue
                                base = 0
                                sum_t = ps2m.tile([128, 512], F32, tag="sum")
                                ssl = sum_t[base : base + 1, :]
                                if probe == "nochain":
                                    for j in range(nblk):
                                        v = j - 4 * c
                                        off = 128 * v if v > 0 else 0
                                        nc.tensor.matmul(
                                            ssl[:, off:512],
                                            lhsT=ones_bf,
                                            rhs=pT(j)[:, off:512],
                                            start=(j == 0),
                                            stop=(j == nblk - 1),
                                        )
                                    rb = rbp.tile([128, 512], F32, tag="rb")
                                    nc.vector.memset(rb, 0.01)
                                    ot = otp.tile(
                                        [128, 512], BF16, tag=f"ot{h}",
                                        name=f"ot_{h}_{c}",
                                    )
                                    nc.vector.tensor_tensor(ot, ps_o, rb, MUL)
                                    ot_c[h] = ot
                                    if 3 in phases:
                                        do_p3(2)
                                    continue
                                for j in range(nblk):
                                    v = j - 4 * c
                                    off = 128 * v if v > 0 else 0
                                    nc.tensor.matmul(
                                        ssl[:, off:512],
                                        lhsT=ones_bf,
                                        rhs=pT(j)[:, off:512],
                                        start=(j == 0),
                                        stop=(j == nblk - 1),
                                    )
                                rsl = rs_all[base : base + 1, :]
                                if bcast == "pe":
                                    # reciprocal in bf16, broadcast across
                                    # partitions via a 1-row PE outer product
                                    # into the (already-read) sum bank
                                    rs_bf = rs_all_bf[base : base + 1, :]
                                    with nc.allow_low_precision(
                                        reason="bf16 softmax reciprocal: ~0.4% "
                                        "common-mode scale error, well inside "
                                        "the 2e-2 budget"
                                    ):
                                        nc.vector.reciprocal(rs_bf, ssl)
                                    rb = ps2m.tile([128, 512], F32, tag="sum")
                                    nc.tensor.matmul(
                                        rb, lhsT=ones_row, rhs=rs_bf,
                                        start=True, stop=True,
                                    )
                                    ot = otp.tile(
                                        [128, 512], BF16, tag=f"ot{h}",
                                        name=f"ot_{h}_{c}",
                                    )
                                    nc.vector.tensor_tensor(ot, ps_o, rb, MUL)
                                    ot_c[h] = ot
                                    if 3 in phases:
                                        do_p3(2)
                                    continue
                                nc.vector.reciprocal(rsl, ssl)
                                rb = rbp.tile([128, 512], F32, tag="rb")
                                if bcast == "pool":
                                    nc.gpsimd.partition_broadcast(rb, rsl)
                                else:
                                    rbx = dram.tile(
                                        [1, 512], F32, tag=f"rbx{h % 3}",
                                        name=f"rbx_{h % 3}",
                                    )
                                    nc.sync.dma_start(rbx, rsl)
                                    nc.gpsimd.dma_start(
                                        rb, rbx[0].partition_broadcast(128)
                                    )
                                # delay the normalize one slot so the Pool
                                # broadcast never blocks the DVE queue
                                pend_ot.append((h, ps_o, rb))
                                if len(pend_ot) > ot_lag:
                                    ph, p_pso, p_rb = pend_ot.pop(0)
                                    ot = otp.tile(
                                        [128, 512], BF16, tag=f"ot{ph}",
                                        name=f"ot_{ph}_{c}",
                                    )
                                    nc.vector.tensor_tensor(ot, p_pso, p_rb, MUL)
                                    ot_c[ph] = ot
                                    if 3 in phases:
                                        do_p3(2)

                            while pend_ot:
                                ph, p_pso, p_rb = pend_ot.pop(0)
                                ot = otp.tile(
                                    [128, 512], BF16, tag=f"ot{ph}",
                                    name=f"ot_{ph}_{c}",
                                )
                                nc.vector.tensor_tensor(ot, p_pso, p_rb, MUL)
                                ot_c[ph] = ot
                                if 3 in phases:
                                    do_p3(2)

                            ot_chunks[c] = ot_c
                            if 3 in phases:
                                pending.extend((c, i) for i in range(CT))
                        if 3 in phases:
                            do_p3(len(pending))
                kvp_ctx.__exit__(None, None, None)

    nc.compile()
    return nc


USE_V3 = True
_BUILD_V2 = build_nc


def build_nc(phases=(1, 2, 3), reps=1, **kw):
    if USE_V3:
        return build_nc_v3(phases=phases, reps=reps, **kw)
    return _BUILD_V2(phases=phases, reps=reps)


_NC_CACHE = None


def _get_nc():
    global _NC_CACHE
    if _NC_CACHE is None:
        _NC_CACHE = build_nc()
    return _NC_CACHE


def _prep_inputs(x, Wqkv, bqkv, Wproj):
    """Host-side shard + pre-tile + bf16 conversion. Returns list of 8 in_maps,
    core index = g * B + b."""
    bf16 = mybir.dt.np(BF16)
    x = np.asarray(x, dtype=np.float32)
    Wqkv = np.asarray(Wqkv, dtype=np.float32)
    Wproj = np.asarray(Wproj, dtype=np.float32)
    bqkv = np.asarray(bqkv, dtype=np.float32)

    # tri[p, f] = 1 if f >= p else 0 (keep keys p <= query f on diag blocks)
    p = np.arange(128)[:, None]
    f = np.arange(128)[None, :]
    tri = np.ascontiguousarray((f >= p).astype(bf16))

    # xT tiles per batch: [128, CT, T] with [p, o, t] = x[b, t, o*128+p]
    xT_b = []
    for b in range(B):
        xt = x[b].T  # [C, T]
        xT_b.append(
            np.ascontiguousarray(
                xt.reshape(CT, 128, T).transpose(1, 0, 2).astype(bf16)
            )
        )

    in_maps = [None] * (G * B)
    for g in range(G):
        cols = np.concatenate(
            [
                np.arange(g * 1024, (g + 1) * 1024),
                np.arange(C + g * 1024, C + (g + 1) * 1024),
                np.arange(2 * C + g * 1024, 2 * C + (g + 1) * 1024),
            ]
        )
        wg = Wqkv[:, cols]  # [C, 3072] = [(o p), (j m)]
        # -> [128 p, 24 j, 16 o, 128 m]
        wg_t = np.ascontiguousarray(
            wg.reshape(CT, 128, NJ, 128).transpose(1, 2, 0, 3).astype(bf16)
        )
        bg = bqkv[cols]  # [3072]
        bg_t = np.ascontiguousarray(bg.reshape(NJ, 128).T)  # [128, 24]
        wp = Wproj[g * 1024 : (g + 1) * 1024, :]  # [1024, C] = [(h p), (i m)]
        # -> [128 p, 16 i, 8 h, 128 m]
        wp_t = np.ascontiguousarray(
            wp.reshape(HPC, 128, CT, 128).transpose(1, 2, 0, 3).astype(bf16)
        )
        for b in range(B):
            in_maps[g * B + b] = dict(
                xT=xT_b[b], wqkv=wg_t, wproj=wp_t, bqkv=bg_t, tri=tri
            )
    return in_maps


def kernel(x, Wqkv, bqkv, Wproj, bproj):
    x = np.asarray(x)
    nc = _get_nc()
    in_maps = _prep_inputs(x, Wqkv, bqkv, Wproj)
    res = run_bass_kernel_spmd(nc, in_maps, core_ids=list(range(G * B)))
    y = np.empty((B, T, C), dtype=np.float32)
    bp = np.asarray(bproj, dtype=np.float32)
    if USE_V3:
        # v3 computes V without its qkv bias; since softmax rows sum to 1,
        # the v-bias contributes exactly bqkv_v @ Wproj to every output row.
        bp = bp + np.asarray(bqkv, dtype=np.float32)[2 * C :] @ np.asarray(
            Wproj, dtype=np.float32
        )
    for b in range(B):
        acc = res.results[b]["yT"].astype(np.float32).copy()
        for g in range(1, G):
            acc += res.results[g * B + b]["yT"]
        y[b] = acc.T + bp[None, :]
    return y

